# revision 69
# baseline (speedup 1.0000x reference)
"""Multi-head causal self-attention (B=2, L=2048, D=1024, H=16) on 8 TRN2
NeuronCores.  ~218 us HW exec (baseline 273-315 us).

Sharding: core c handles batch b = c // 4 and head group g = c % 4 (4 heads,
i.e. a 256-wide slice of the QKV output dim and the matching 256 rows of
Wo^T).  Each core computes a full (L, D) partial of the output projection;
the host sums the 4 partials per batch and adds bo.

All matmul operands are bf16/f16 (accumulation f32 in PSUM): vs the f32r
original this halves LDWEIGHTS (which was rate-limiting at 229 ns vs the
213 ns per-512-column matmul stream) and makes every matmul stream at the
full 2.4 GHz / 1 column/cycle.

On-core layout:
  XT  [128, 8, 2048]   x^T (d-chunk on partitions).  x tiles 0-3 transpose
                       on the PE during the startup DMA window (also ramps
                       the HAM clock); tiles 4-15 go f32 load -> Pool cast
                       to bf16 -> XBAR DMA transpose (14 ns/16x128 tile on
                       the otherwise-idle DMA engines), software-pipelined
                       one 512-row block ahead of consumption.
  WqT/WkT/WvT [128, 8, 256], WoT [128, 2, 1024]  W^T via PE transposes in
                       the same startup window.
  QT  [128, 2, 2048]   q^T (dq on partitions, chunk = head pair)
  KTz [128, 4, 2048]   k^T zero-padded per head to K=128 rows: the PE HAM
                       clock gate only un-throttles (1.2 -> 2.4 GHz) when
                       matmuls stream all 128 partitions.
  Vp  [128, 4, 4, 65]  v natural + ones column (softmax denominator trick),
                       built from 512-col v^T matmuls (weight loads stay
                       hidden) + f16 PE transposes back to natural layout.
  OT  [128, 2, 2048]   attention out^T, normalized in place

DMA choreography matters as much as compute: one HWDGE ring's transfers
are serial in issue order, so the sync ring is hand-ordered (x0-3 + w
loads, then per-block xbar transposes ahead of the next block's loads,
y stores last) and the tiny bias loads ride the scalar ring.

Projections and attention are interleaved per 512-row q block.  Attention
per (qt, head): s^T[k, q] = KTz_h . QT_pair; exp on ACT from a 2-bank PSUM
pair (ACT is the attention-phase pacer at ~89 us total); causal mask via a
DVE multiply with a precomputed 128x128 triangle (keeps the Pool engine
off the exp -> AV latency chain), with AV column-trimmed to the valid q
range on diagonal tiles; o^T + denominator accumulated in PSUM with V';
normalize = PE ones-broadcast of the denominator + DVE
reciprocal_approx_fast (5x faster than reciprocal(); the denominator is a
sum of exps, far from the undefined edge cases) + one multiply, emitted
one tile late so the PE stream never waits.  The output projection is
woven in per 512-row q block, and the last block's is split by
contraction half so only half of it (plus adds and stores) trails the
final normalize.

Things measured NOT to work here: f32r anywhere (LDWEIGHTS-bound),
software-DGE cast DMAs (~7x slower than HWDGE), loads on the scalar ring
(starved), fp8 DoubleRow AV (mixing DoubleRow and normal matmuls in one
PSUM accumulation group corrupts it -> NaN; pure-DR works but early
causal rows then fail the precision gate), scalar-engine psum->sbuf
copies during flight (ACT in-order execution stalls the exp stream), and
causal triangle trimming of scores/exp (short matmul streams expose
LDWEIGHTS and break the pair-pipeline rhythm).
"""

import sys

for _p in ("/opt/trn_rl_repo", "/root/.axon_site/_ro/trn_rl_repo"):
    if _p not in sys.path:
        sys.path.append(_p)

from contextlib import ExitStack

import numpy as np

import concourse.bass as bass
import concourse.tile as tile
from concourse import bacc, mybir
from concourse.bass_utils import run_bass_kernel_spmd
from concourse.masks import make_identity

F32 = mybir.dt.float32
F32R = mybir.dt.float32r
F16 = mybir.dt.float16
BF16 = mybir.dt.bfloat16
F8E4 = mybir.dt.float8e4

B, L, D, H = 2, 2048, 1024, 16
DK = D // H  # 64
NCORES = 8
GH = 4  # heads per core
C = GH * DK  # 256: per-core slice of the qkv/head dim
QT_TILES = L // 512  # 4
KT_TILES = L // 128  # 16
DCH = D // 128  # 8


def _build_program():
    nc = bacc.Bacc("TRN2", target_bir_lowering=False, debug=False, num_devices=NCORES)

    x_d = nc.dram_tensor("x", [L, D], F32, kind="ExternalInput").ap()
    wq_d = nc.dram_tensor("wq", [C, D], F32, kind="ExternalInput").ap()
    wk_d = nc.dram_tensor("wk", [C, D], F32, kind="ExternalInput").ap()
    wv_d = nc.dram_tensor("wv", [C, D], F32, kind="ExternalInput").ap()
    wo_d = nc.dram_tensor("wo", [D, C], F32, kind="ExternalInput").ap()
    bq_d = nc.dram_tensor("bq", [C], F32, kind="ExternalInput").ap()
    bk_d = nc.dram_tensor("bk", [C], F32, kind="ExternalInput").ap()
    bv_d = nc.dram_tensor("bv", [C], F32, kind="ExternalInput").ap()
    out_d = nc.dram_tensor("out", [L, D], F32, kind="ExternalOutput").ap()

    with tile.TileContext(nc) as tc, ExitStack() as ctx:
        pool = ctx.enter_context(tc.tile_pool(name="persist", bufs=1))
        psum = ctx.enter_context(tc.tile_pool(name="psum", bufs=2, space="PSUM"))
        psum_o = ctx.enter_context(tc.tile_pool(name="psum_o", bufs=2, space="PSUM"))
        psum2 = ctx.enter_context(tc.tile_pool(name="psum2", bufs=2, space="PSUM"))
        cp = ctx.enter_context(tc.tile_pool(name="copies", bufs=3))
        yp = ctx.enter_context(tc.tile_pool(name="youts", bufs=2))

        ones_f32 = pool.tile([1, 128], F32)
        nc.gpsimd.memset(ones_f32[:], 1.0)
        ones_r = pool.tile([1, 128], F32R)
        nc.vector.tensor_copy(ones_r[:], ones_f32[:])
        ident = pool.tile([128, 128], F32)
        make_identity(nc, ident)
        ident16 = pool.tile([128, 128], F16)
        nc.vector.tensor_copy(ident16[:], ident[:])
        tri_mask = pool.tile([128, 128], F16)
        nc.gpsimd.memset(tri_mask[:], 1.0)
        nc.gpsimd.affine_select(
            out=tri_mask[:],
            in_=tri_mask[:],
            pattern=[[1, 128]],
            compare_op=mybir.AluOpType.is_ge,
            fill=0.0,
            base=0,
            channel_multiplier=-1,
        )

        bq_sb = pool.tile([128, 2], F32)
        nc.scalar.dma_start(bq_sb[:], bq_d.rearrange("(c p) -> p c", p=128))
        bk_sb = pool.tile([128, 2], F32)
        nc.scalar.dma_start(bk_sb[:], bk_d.rearrange("(c p) -> p c", p=128))
        bv_sb = pool.tile([128, 2], F32)
        nc.scalar.dma_start(bv_sb[:], bv_d.rearrange("(c p) -> p c", p=128))

        QTs = [pool.tile([128, 2, 512], F16, name=f"QT{g}") for g in range(4)]
        KTzs = [pool.tile([128, GH, 512], F16, name=f"KTz{g}") for g in range(4)]
        Vp16 = [pool.tile([128, 4, GH, DK + 1], F16, name=f"Vp16_{g}") for g in range(4)]
        OTs = [pool.tile([128, 2, 512], BF16, name=f"OT{g}") for g in range(4)]

        # ---- weight + first-block x transposes on the PE (f32 in, psum,
        # DVE copy casts to bf16) during the startup DMA window; x4-15 go
        # HWDGE f32 load -> DVE cast -> XBAR DMA transpose on the sync
        # ring so the PE never pays for them. ----
        XT = pool.tile([128, DCH, L], BF16)
        WT = {}
        lp = ctx.enter_context(tc.tile_pool(name="loads", bufs=6))
        cb = ctx.enter_context(tc.tile_pool(name="casts", bufs=3))
        xsbs, xbs = {}, {}

        def load_x(qt, split=False):
            # split=True loads in two halves so the PE transpose of the
            # first 4 d-chunks can start as soon as half the tile lands
            xsb = lp.tile([128, D], F32, tag="xsb")
            if split:
                for hh in range(2):
                    nc.sync.dma_start(
                        xsb[:, hh * 512 : (hh + 1) * 512],
                        x_d[qt * 128 : (qt + 1) * 128, hh * 512 : (hh + 1) * 512],
                    )
            else:
                nc.sync.dma_start(xsb[:], x_d[qt * 128 : (qt + 1) * 128, :])
            xsbs[qt] = xsb

        def cast_x(qt):
            # Pool engine: ~2.5x slower per copy than DVE but completely off
            # the DVE relay chain (proj psum -> QT/KTz -> scores) that paces
            # each block boundary; transposes still land with slack to spare
            xb = cb.tile([128, D], BF16, tag="xb")
            nc.gpsimd.tensor_copy(xb[:], xsbs.pop(qt)[:])
            xbs[qt] = xb

        def xpose_x(qt):
            nc.sync.dma_start(
                XT[:, :, qt * 128 : (qt + 1) * 128], xbs.pop(qt)[:], transpose=True
            )

        wsbs = {}

        def load_w(name, w_d):
            wsb = lp.tile([128, 2048], F32, tag="wsb", bufs=4)
            nc.sync.dma_start(
                wsb[:].rearrange("p (c d) -> p c d", c=w_d.shape[0] // 128),
                w_d.rearrange("(c p) d -> p c d", p=128),
            )
            wsbs[name] = wsb

        def _transpose_block(dst, src, n_chunks, col0):
            """PE-transpose n_chunks 128x128 f32 blocks of src into
            dst[:, chunk, col0:col0+128] (DVE copy casts to dst dtype)."""
            for half in range(0, n_chunks, 4):
                n = min(4, n_chunks - half)
                pt = psum.tile([128, 512], F32, tag="ps", name="pt")
                for i in range(n):
                    dci = half + i
                    nc.tensor.matmul(
                        pt[:, i * 128 : (i + 1) * 128],
                        lhsT=src[:, dci * 128 : (dci + 1) * 128],
                        rhs=ident[:],
                        is_transpose=True,
                        start=(i == 0),
                        stop=(i == n - 1),
                    )
                nc.vector.tensor_copy(
                    dst[:, half : half + n, col0 : col0 + 128],
                    pt[:, : n * 128].rearrange("p (c q) -> p c q", c=n),
                )

        def xpose_pe_x(qt):
            _transpose_block(XT, xsbs.pop(qt)[:], DCH, qt * 128)

        def wt_pe(name, dst, outer):
            # wsb [128, outer, inner*128]; chunk r transposes into
            # dst[:, :, r*128:(r+1)*128]
            wsb = wsbs.pop(name)[:].rearrange("p (c d) -> p c d", c=outer)
            for r in range(outer):
                _transpose_block(dst, wsb[:, r, :], 2048 // (128 * outer), r * 128)

        # prologue: x0-3 + wq/wk loads, PE transposes fill the startup
        for qt in range(4):
            load_x(qt, split=True)
        load_w("q", wq_d)
        load_w("k", wk_d)
        for name in ("q", "k", "v"):
            WT[name] = pool.tile([128, DCH, C], BF16, name=f"W{name}T")
        WoT = pool.tile([128, 2, D], BF16)
        for qt in range(4):
            xpose_pe_x(qt)
        wt_pe("q", WT["q"], 2)
        wt_pe("k", WT["k"], 2)
        load_w("v", wv_d)
        for qt in range(4, 6):
            load_x(qt)
        load_w("o", wo_d)
        for qt in range(6, 8):
            load_x(qt)
        # zero-fills: Pool memsets + small DVE copies, off the cast path
        for g in range(4):
            for h in range(GH):
                zp = 64 - 64 * (h % 2)
                nc.gpsimd.memset(KTzs[g][zp : zp + 64, h, :], 0.0)
        for g in range(4):
            nc.gpsimd.memset(Vp16[g][:, :, :, DK], 1.0)

        with nc.allow_low_precision(reason="bf16/f16 matmul inputs"):

            def qk_proj(blk):
                # j-major, k first: attention h0 needs only the k-j0 and
                # q-j0 outputs, so it can start after two psums, not three
                for j in range(2):
                    for name in ("k", "q"):
                        wt = WT[name]
                        b_sb = bq_sb if name == "q" else bk_sb
                        ps = psum.tile([128, 512], F32, tag="ps")
                        for dci in range(DCH):
                            nc.tensor.matmul(
                                ps[:],
                                lhsT=wt[:, dci, j * 128 : (j + 1) * 128],
                                rhs=XT[:, dci, blk * 512 : (blk + 1) * 512],
                                start=(dci == 0),
                                stop=(dci == DCH - 1),
                            )
                        if name == "q":
                            nc.vector.tensor_tensor(
                                QTs[blk][:, j, :],
                                ps[:],
                                b_sb[:, j, None].to_broadcast((128, 512)),
                                mybir.AluOpType.add,
                            )
                        else:
                            # write k^T split into the zero-padded KTz
                            for half in range(2):
                                hp = 64 * half
                                nc.vector.tensor_tensor(
                                    KTzs[blk][hp : hp + 64, 2 * j + half, :],
                                    ps[hp : hp + 64, :],
                                    b_sb[hp : hp + 64, j, None].to_broadcast(
                                        (64, 512)
                                    ),
                                    mybir.AluOpType.add,
                                )

            def v_proj(blk):
                # v^T via 512-col matmuls (weight loads stay hidden), bias
                # added per-partition on the psum read, then f16 PE
                # transposes back into the natural-layout Vp
                vt = cb.tile([128, 2, 512], F16, tag="vt", bufs=2)
                for ch in range(2):
                    ps = psum.tile([128, 512], F32, tag="ps")
                    for dci in range(DCH):
                        nc.tensor.matmul(
                            ps[:],
                            lhsT=WT["v"][:, dci, ch * 128 : (ch + 1) * 128],
                            rhs=XT[:, dci, blk * 512 : (blk + 1) * 512],
                            start=(dci == 0),
                            stop=(dci == DCH - 1),
                        )
                    nc.vector.tensor_tensor(
                        vt[:, ch, :],
                        ps[:],
                        bv_sb[:, ch, None].to_broadcast((128, 512)),
                        mybir.AluOpType.add,
                    )
                for lsub in range(4):
                    pv = psum.tile([128, 256], F16, tag="ps")
                    for ch in range(2):
                        nc.tensor.matmul(
                            pv[:, ch * 128 : (ch + 1) * 128],
                            lhsT=vt[:, ch, lsub * 128 : (lsub + 1) * 128],
                            rhs=ident16[:],
                            is_transpose=True,
                            start=(ch == 0),
                            stop=(ch == 1),
                        )
                    nc.vector.tensor_copy(
                        Vp16[blk][:, lsub, :, 0:DK],
                        pv[:].rearrange("p (h d) -> p h d", h=GH),
                    )

            def normalize(h, qt, pso):
                hj, hp = h // 2, 64 * (h % 2)
                den_r = cp.tile([1, 512], F32R, tag="den", bufs=2)
                nc.vector.tensor_copy(den_r[:], pso[64:65, :])
                psb = psum.tile([128, 512], F32, tag="ps")
                nc.tensor.matmul(
                    psb[:64], lhsT=ones_r[:, 0:64], rhs=den_r[:], start=True, stop=True
                )
                rb = cp.tile([64, 512], F32, tag="rb", bufs=2)
                nc.vector.reciprocal_approx_fast(rb[:], psb[:64])
                nc.vector.tensor_tensor(
                    OTs[qt][hp : hp + 64, hj, :],
                    pso[:64],
                    rb[:],
                    mybir.AluOpType.mult,
                )

            def outproj(qt512):
                # project q rows [qt512*512, +512) and DMA them out; woven
                # into the next q-tile's attention so PE stays dense and the
                # output DMA is spread across the kernel.
                for sub in range(4):
                    q0 = qt512 * 512 + sub * 128
                    for e in range(2):
                        psy = psum.tile([128, 512], F32, tag="ps")
                        for cj in range(2):
                            nc.tensor.matmul(
                                psy[:],
                                lhsT=OTs[qt512][:, cj, sub * 128 : (sub + 1) * 128],
                                rhs=WoT[:, cj, e * 512 : (e + 1) * 512],
                                start=(cj == 0),
                                stop=(cj == 1),
                            )
                        y_sb = yp.tile([128, 512], F32, tag="y")
                        nc.vector.tensor_copy(y_sb[:], psy[:])
                        nc.sync.dma_start(
                            out_d[q0 : q0 + 128, e * 512 : (e + 1) * 512], y_sb[:]
                        )

            y3 = {}
            y8 = ctx.enter_context(tc.tile_pool(name="ytail", bufs=8))

            def outproj_half(qt512, cj):
                # last-block outproj split by contraction half: cj=0 runs
                # during the final attention head (PE otherwise waits on
                # exp), cj=1 + accumulate + DMA is all that's left after
                # the last normalize.
                for sub in range(4):
                    for e in range(2):
                        idx = sub * 2 + e
                        psy = psum.tile([128, 512], F32, tag="ps")
                        nc.tensor.matmul(
                            psy[:],
                            lhsT=OTs[qt512][:, cj, sub * 128 : (sub + 1) * 128],
                            rhs=WoT[:, cj, e * 512 : (e + 1) * 512],
                            start=True,
                            stop=True,
                        )
                        if cj == 0:
                            y_sb = y8.tile([128, 512], F32, tag="y8")
                            nc.vector.tensor_copy(y_sb[:], psy[:])
                            y3[idx] = y_sb
                        else:
                            y_sb = y3[idx]
                            nc.vector.tensor_tensor(
                                y_sb[:], y_sb[:], psy[:], mybir.AluOpType.add
                            )
                            q0 = qt512 * 512 + sub * 128
                            nc.sync.dma_start(
                                out_d[q0 : q0 + 128, e * 512 : (e + 1) * 512], y_sb[:]
                            )

            pending = None

            def attn(qt):
                nonlocal pending
                n_kt = 4 * qt + 4
                for h in range(GH):
                    hj = h // 2
                    pso = psum_o.tile([128, 512], F32, tag="pso")
                    for kt2 in range(n_kt // 2):  # k-tile pairs share a
                        pss = psum2.tile([128, 1024], F32, tag="ps2")  # 2-bank psum
                        for i in range(2):
                            kt = 2 * kt2 + i
                            nc.tensor.matmul(
                                pss[:, i * 512 : (i + 1) * 512],
                                lhsT=KTzs[kt // 4][
                                    :, h, (kt % 4) * 128 : (kt % 4 + 1) * 128
                                ],
                                rhs=QTs[qt][:, hj, :],
                                start=True,
                                stop=True,
                            )
                        p_sb = cp.tile([128, 1024], F16, tag="p", bufs=5)
                        # one activation per pair, but start at the first
                        # valid column of the pair's first k-tile: for the
                        # (i=2,3) diagonal pair this trims 256 columns while
                        # keeping the single-exp pipeline rhythm (the
                        # mid-range garbage it still covers is never read)
                        e0 = 128 * max(0, 2 * kt2 - 4 * qt)
                        nc.scalar.activation(
                            p_sb[:, e0:],
                            pss[:, e0:],
                            mybir.ActivationFunctionType.Exp,
                            scale=0.125,
                        )
                        for i in range(2):
                            kt = 2 * kt2 + i
                            if kt >= 4 * qt:  # diagonal tile: causal mask via
                                # a DVE multiply (keeps the Pool engine off
                                # the exp -> AV latency chain)
                                d0 = (kt - 4 * qt) * 128
                                nc.vector.tensor_tensor(
                                    p_sb[:, i * 512 + d0 : i * 512 + d0 + 128],
                                    p_sb[:, i * 512 + d0 : i * 512 + d0 + 128],
                                    tri_mask[:],
                                    mybir.AluOpType.mult,
                                )
                            # AV restricted to valid q columns; cols below
                            # the diagonal tile are never read so the exp
                            # garbage there is harmless
                            d0 = max(0, (kt - 4 * qt) * 128)
                            nc.tensor.matmul(
                                pso[:65, d0:],
                                lhsT=Vp16[kt // 4][:, kt % 4, h, :],
                                rhs=p_sb[:, i * 512 + d0 : (i + 1) * 512],
                                start=(kt == 0),
                                stop=(kt == n_kt - 1),
                                skip_group_check=True,
                            )
                        if kt2 == 0 and pending is not None:
                            normalize(*pending)  # previous tile, PE has work
                            pending = None
                    pending = (h, qt, pso)
                    if h == 0 and qt > 0:
                        outproj(qt - 1)  # previous q block fully normalized
                    if h == 2 and qt == QT_TILES - 1:
                        outproj_half(qt, 0)  # heads 0/1 already normalized

            # ==== interleaved: per 512-row block, projections then attention,
            # with the next blocks' x load/cast/transpose woven in
            for blk in range(QT_TILES):
                # next block's x casts + XBAR transposes first: the casts
                # only wait on loads, so DVE drains them before the proj
                # psum reads and the transposes land well before block+1
                if blk < 3:
                    for qt in range(4 + 4 * blk, 8 + 4 * blk):
                        cast_x(qt)
                    for qt in range(4 + 4 * blk, 8 + 4 * blk):
                        xpose_x(qt)
                qk_proj(blk)
                if blk == 0:
                    wt_pe("v", WT["v"], 2)
                if blk == 1:
                    wt_pe("o", WoT, 8)
                v_proj(blk)
                if blk < 2:
                    for qt in range(8 + 4 * blk, 12 + 4 * blk):
                        load_x(qt)
                attn(blk)
            normalize(*pending)
            outproj_half(QT_TILES - 1, 1)

    nc.compile()
    return nc


_NC_CACHE = None


def _get_program():
    global _NC_CACHE
    if _NC_CACHE is None:
        _NC_CACHE = _build_program()
    return _NC_CACHE


def _run(in_maps, trace=False, **kw):
    nc = _get_program()
    return run_bass_kernel_spmd(nc, in_maps, list(range(NCORES)), trace=trace, **kw)


def _make_in_maps(x, Wq, bq, Wk, bk, Wv, bv, Wo, bo):
    a = lambda v: np.ascontiguousarray(np.asarray(v, dtype=np.float32))
    in_maps = []
    for core in range(NCORES):
        b, g = divmod(core, 4)
        s = slice(g * C, (g + 1) * C)
        in_maps.append(
            {
                "x": a(x[b]),
                "wq": a(Wq[s, :]),
                "wk": a(Wk[s, :]),
                "wv": a(Wv[s, :]),
                "wo": a(Wo[:, s]),
                "bq": a(bq[s]),
                "bk": a(bk[s]),
                "bv": a(bv[s]),
            }
        )
    return in_maps


def kernel(x, Wq, bq, Wk, bk, Wv, bv, Wo, bo, _trace=False, _trace_out=None, _tmpdir=None):
    in_maps = _make_in_maps(x, Wq, bq, Wk, bk, Wv, bv, Wo, bo)
    res = _run(in_maps, trace=_trace, tmpdir=_tmpdir)
    if _trace_out is not None:
        _trace_out.append(res)
    bo = np.asarray(bo, dtype=np.float32)
    out = np.empty((B, L, D), dtype=np.float32)
    for b in range(B):
        acc = res.results[4 * b]["out"].astype(np.float32)
        for g in range(1, 4):
            acc = acc + res.results[4 * b + g]["out"]
        out[b] = acc + bo[None, :]
    return out


# revision 71
# speedup vs baseline: 1.0256x; 1.0256x over previous
"""Multi-head causal self-attention (B=2, L=2048, D=1024, H=16) on 8 TRN2
NeuronCores.  ~218 us HW exec (baseline 273-315 us).

Sharding: core c handles batch b = c // 4 and head group g = c % 4 (4 heads,
i.e. a 256-wide slice of the QKV output dim and the matching 256 rows of
Wo^T).  Each core computes a full (L, D) partial of the output projection;
the host sums the 4 partials per batch and adds bo.

All matmul operands are bf16/f16 (accumulation f32 in PSUM): vs the f32r
original this halves LDWEIGHTS (which was rate-limiting at 229 ns vs the
213 ns per-512-column matmul stream) and makes every matmul stream at the
full 2.4 GHz / 1 column/cycle.

On-core layout:
  XT  [128, 8, 2048]   x^T (d-chunk on partitions).  x tiles 0-3 transpose
                       on the PE during the startup DMA window (also ramps
                       the HAM clock); tiles 4-15 go f32 load -> Pool cast
                       to bf16 -> XBAR DMA transpose (14 ns/16x128 tile on
                       the otherwise-idle DMA engines), software-pipelined
                       one 512-row block ahead of consumption.
  WqT/WkT/WvT [128, 8, 256], WoT [128, 2, 1024]  W^T via PE transposes in
                       the same startup window.
  QT  [128, 2, 2048]   q^T (dq on partitions, chunk = head pair)
  KTz [128, 4, 2048]   k^T zero-padded per head to K=128 rows: the PE HAM
                       clock gate only un-throttles (1.2 -> 2.4 GHz) when
                       matmuls stream all 128 partitions.
  Vp  [128, 4, 4, 65]  v natural + ones column (softmax denominator trick),
                       built from 512-col v^T matmuls (weight loads stay
                       hidden) + f16 PE transposes back to natural layout.
  OT  [128, 2, 2048]   attention out^T, normalized in place

DMA choreography matters as much as compute: one HWDGE ring's transfers
are serial in issue order, so the sync ring is hand-ordered (x0-3 + w
loads, then per-block xbar transposes ahead of the next block's loads,
y stores last) and the tiny bias loads ride the scalar ring.

Projections and attention are interleaved per 512-row q block.  Attention
per (qt, head): s^T[k, q] = KTz_h . QT_pair; exp on ACT from a 2-bank PSUM
pair (ACT is the attention-phase pacer at ~89 us total); causal mask via a
DVE multiply with a precomputed 128x128 triangle (keeps the Pool engine
off the exp -> AV latency chain), with AV column-trimmed to the valid q
range on diagonal tiles; o^T + denominator accumulated in PSUM with V';
normalize = PE ones-broadcast of the denominator + DVE
reciprocal_approx_fast (5x faster than reciprocal(); the denominator is a
sum of exps, far from the undefined edge cases) + one multiply, emitted
one tile late so the PE stream never waits.  The output projection is
woven in per 512-row q block, and the last block's is split by
contraction half so only half of it (plus adds and stores) trails the
final normalize.

Things measured NOT to work here: f32r anywhere (LDWEIGHTS-bound),
software-DGE cast DMAs (~7x slower than HWDGE), loads on the scalar ring
(starved), fp8 DoubleRow AV (mixing DoubleRow and normal matmuls in one
PSUM accumulation group corrupts it -> NaN; pure-DR works but early
causal rows then fail the precision gate), scalar-engine psum->sbuf
copies during flight (ACT in-order execution stalls the exp stream), and
causal triangle trimming of scores/exp (short matmul streams expose
LDWEIGHTS and break the pair-pipeline rhythm).
"""

import sys

for _p in ("/opt/trn_rl_repo", "/root/.axon_site/_ro/trn_rl_repo"):
    if _p not in sys.path:
        sys.path.append(_p)

from contextlib import ExitStack

import numpy as np

import concourse.bass as bass
import concourse.tile as tile
from concourse import bacc, mybir
from concourse.bass_utils import run_bass_kernel_spmd
from concourse.masks import make_identity

F32 = mybir.dt.float32
F32R = mybir.dt.float32r
F16 = mybir.dt.float16
BF16 = mybir.dt.bfloat16
F8E4 = mybir.dt.float8e4

B, L, D, H = 2, 2048, 1024, 16
DK = D // H  # 64
NCORES = 8
GH = 4  # heads per core
C = GH * DK  # 256: per-core slice of the qkv/head dim
QT_TILES = L // 512  # 4
KT_TILES = L // 128  # 16
DCH = D // 128  # 8


def _build_program():
    nc = bacc.Bacc("TRN2", target_bir_lowering=False, debug=False, num_devices=NCORES)

    x_d = nc.dram_tensor("x", [L, D], F32, kind="ExternalInput").ap()
    wq_d = nc.dram_tensor("wq", [C, D], F32, kind="ExternalInput").ap()
    wk_d = nc.dram_tensor("wk", [C, D], F32, kind="ExternalInput").ap()
    wv_d = nc.dram_tensor("wv", [C, D], F32, kind="ExternalInput").ap()
    wo_d = nc.dram_tensor("wo", [D, C], F32, kind="ExternalInput").ap()
    bq_d = nc.dram_tensor("bq", [C], F32, kind="ExternalInput").ap()
    bk_d = nc.dram_tensor("bk", [C], F32, kind="ExternalInput").ap()
    bv_d = nc.dram_tensor("bv", [C], F32, kind="ExternalInput").ap()
    out_d = nc.dram_tensor("out", [L, D], F32, kind="ExternalOutput").ap()

    with tile.TileContext(nc) as tc, ExitStack() as ctx:
        pool = ctx.enter_context(tc.tile_pool(name="persist", bufs=1))
        psum = ctx.enter_context(tc.tile_pool(name="psum", bufs=2, space="PSUM"))
        psum_o = ctx.enter_context(tc.tile_pool(name="psum_o", bufs=2, space="PSUM"))
        psum2 = ctx.enter_context(tc.tile_pool(name="psum2", bufs=2, space="PSUM"))
        cp = ctx.enter_context(tc.tile_pool(name="copies", bufs=3))
        yp = ctx.enter_context(tc.tile_pool(name="youts", bufs=2))

        ones_f32 = pool.tile([1, 128], F32)
        nc.gpsimd.memset(ones_f32[:], 1.0)
        ones_r = pool.tile([1, 128], F32R)
        nc.vector.tensor_copy(ones_r[:], ones_f32[:])
        ident = pool.tile([128, 128], F32)
        make_identity(nc, ident)
        ident16 = pool.tile([128, 128], F16)
        nc.vector.tensor_copy(ident16[:], ident[:])
        tri_mask = pool.tile([128, 128], F16)
        nc.gpsimd.memset(tri_mask[:], 1.0)
        nc.gpsimd.affine_select(
            out=tri_mask[:],
            in_=tri_mask[:],
            pattern=[[1, 128]],
            compare_op=mybir.AluOpType.is_ge,
            fill=0.0,
            base=0,
            channel_multiplier=-1,
        )

        bq_sb = pool.tile([128, 2], F32)
        nc.scalar.dma_start(bq_sb[:], bq_d.rearrange("(c p) -> p c", p=128))
        bk_sb = pool.tile([128, 2], F32)
        nc.scalar.dma_start(bk_sb[:], bk_d.rearrange("(c p) -> p c", p=128))
        bv_sb = pool.tile([128, 2], F32)
        nc.scalar.dma_start(bv_sb[:], bv_d.rearrange("(c p) -> p c", p=128))

        QTs = [pool.tile([128, 2, 512], F16, name=f"QT{g}") for g in range(4)]
        KTzs = [pool.tile([128, GH, 512], F16, name=f"KTz{g}") for g in range(4)]
        Vp16 = [pool.tile([128, 4, GH, DK + 1], F16, name=f"Vp16_{g}") for g in range(4)]
        OTs = [pool.tile([128, 2, 512], BF16, name=f"OT{g}") for g in range(4)]

        # ---- weight + first-block x transposes on the PE (f32 in, psum,
        # DVE copy casts to bf16) during the startup DMA window; x4-15 go
        # HWDGE f32 load -> DVE cast -> XBAR DMA transpose on the sync
        # ring so the PE never pays for them. ----
        XT = pool.tile([128, DCH, L], BF16)
        WT = {}
        lp = ctx.enter_context(tc.tile_pool(name="loads", bufs=6))
        cb = ctx.enter_context(tc.tile_pool(name="casts", bufs=3))
        xsbs, xbs = {}, {}

        def load_x(qt, split=False):
            # split=True loads in two halves so the PE transpose of the
            # first 4 d-chunks can start as soon as half the tile lands
            xsb = lp.tile([128, D], F32, tag="xsb")
            if split:
                for hh in range(2):
                    nc.sync.dma_start(
                        xsb[:, hh * 512 : (hh + 1) * 512],
                        x_d[qt * 128 : (qt + 1) * 128, hh * 512 : (hh + 1) * 512],
                    )
            else:
                nc.sync.dma_start(xsb[:], x_d[qt * 128 : (qt + 1) * 128, :])
            xsbs[qt] = xsb

        def cast_x(qt):
            # Pool engine: ~2.5x slower per copy than DVE but completely off
            # the DVE relay chain (proj psum -> QT/KTz -> scores) that paces
            # each block boundary; transposes still land with slack to spare
            xb = cb.tile([128, D], BF16, tag="xb")
            nc.gpsimd.tensor_copy(xb[:], xsbs.pop(qt)[:])
            xbs[qt] = xb

        def xpose_x(qt):
            nc.sync.dma_start(
                XT[:, :, qt * 128 : (qt + 1) * 128], xbs.pop(qt)[:], transpose=True
            )

        wsbs = {}

        def load_w(name, w_d):
            wsb = lp.tile([128, 2048], F32, tag="wsb", bufs=4)
            nc.sync.dma_start(
                wsb[:].rearrange("p (c d) -> p c d", c=w_d.shape[0] // 128),
                w_d.rearrange("(c p) d -> p c d", p=128),
            )
            wsbs[name] = wsb

        def _transpose_block(dst, src, n_chunks, col0):
            """PE-transpose n_chunks 128x128 f32 blocks of src into
            dst[:, chunk, col0:col0+128] (DVE copy casts to dst dtype)."""
            for half in range(0, n_chunks, 4):
                n = min(4, n_chunks - half)
                pt = psum.tile([128, 512], F32, tag="ps", name="pt")
                for i in range(n):
                    dci = half + i
                    nc.tensor.matmul(
                        pt[:, i * 128 : (i + 1) * 128],
                        lhsT=src[:, dci * 128 : (dci + 1) * 128],
                        rhs=ident[:],
                        is_transpose=True,
                        start=(i == 0),
                        stop=(i == n - 1),
                    )
                nc.vector.tensor_copy(
                    dst[:, half : half + n, col0 : col0 + 128],
                    pt[:, : n * 128].rearrange("p (c q) -> p c q", c=n),
                )

        def xpose_pe_x(qt):
            _transpose_block(XT, xsbs.pop(qt)[:], DCH, qt * 128)

        def wt_pe(name, dst, outer):
            # wsb [128, outer, inner*128]; chunk r transposes into
            # dst[:, :, r*128:(r+1)*128]
            wsb = wsbs.pop(name)[:].rearrange("p (c d) -> p c d", c=outer)
            for r in range(outer):
                _transpose_block(dst, wsb[:, r, :], 2048 // (128 * outer), r * 128)

        # prologue: x0-3 + wq/wk loads, PE transposes fill the startup
        for qt in range(4):
            load_x(qt, split=True)
        load_w("q", wq_d)
        load_w("k", wk_d)
        for name in ("q", "k", "v"):
            WT[name] = pool.tile([128, DCH, C], BF16, name=f"W{name}T")
        WoT = pool.tile([128, 2, D], BF16)
        for qt in range(4):
            xpose_pe_x(qt)
        wt_pe("q", WT["q"], 2)
        wt_pe("k", WT["k"], 2)
        load_w("v", wv_d)
        for qt in range(4, 6):
            load_x(qt)
        load_w("o", wo_d)
        for qt in range(6, 8):
            load_x(qt)
        # zero-fills: Pool memsets + small DVE copies, off the cast path
        for g in range(4):
            for h in range(GH):
                zp = 64 - 64 * (h % 2)
                nc.gpsimd.memset(KTzs[g][zp : zp + 64, h, :], 0.0)
        for g in range(4):
            nc.gpsimd.memset(Vp16[g][:, :, :, DK], 1.0)

        with nc.allow_low_precision(reason="bf16/f16 matmul inputs"):

            def qk_proj(blk):
                # j-major, k first: attention h0 needs only the k-j0 and
                # q-j0 outputs, so it can start after two psums, not three
                for j in range(2):
                    for name in ("k", "q"):
                        wt = WT[name]
                        b_sb = bq_sb if name == "q" else bk_sb
                        ps = psum.tile([128, 512], F32, tag="ps")
                        for dci in range(DCH):
                            nc.tensor.matmul(
                                ps[:],
                                lhsT=wt[:, dci, j * 128 : (j + 1) * 128],
                                rhs=XT[:, dci, blk * 512 : (blk + 1) * 512],
                                start=(dci == 0),
                                stop=(dci == DCH - 1),
                            )
                        if name == "q":
                            nc.vector.tensor_tensor(
                                QTs[blk][:, j, :],
                                ps[:],
                                b_sb[:, j, None].to_broadcast((128, 512)),
                                mybir.AluOpType.add,
                            )
                        else:
                            # write k^T split into the zero-padded KTz
                            for half in range(2):
                                hp = 64 * half
                                nc.vector.tensor_tensor(
                                    KTzs[blk][hp : hp + 64, 2 * j + half, :],
                                    ps[hp : hp + 64, :],
                                    b_sb[hp : hp + 64, j, None].to_broadcast(
                                        (64, 512)
                                    ),
                                    mybir.AluOpType.add,
                                )

            def v_proj(blk):
                # v^T via 512-col matmuls (weight loads stay hidden), bias
                # added per-partition on the psum read, then f16 PE
                # transposes back into the natural-layout Vp
                vt = cb.tile([128, 2, 512], F16, tag="vt", bufs=2)
                for ch in range(2):
                    ps = psum.tile([128, 512], F32, tag="ps")
                    for dci in range(DCH):
                        nc.tensor.matmul(
                            ps[:],
                            lhsT=WT["v"][:, dci, ch * 128 : (ch + 1) * 128],
                            rhs=XT[:, dci, blk * 512 : (blk + 1) * 512],
                            start=(dci == 0),
                            stop=(dci == DCH - 1),
                        )
                    nc.vector.tensor_tensor(
                        vt[:, ch, :],
                        ps[:],
                        bv_sb[:, ch, None].to_broadcast((128, 512)),
                        mybir.AluOpType.add,
                    )
                for lsub in range(4):
                    pv = psum.tile([128, 256], F16, tag="ps")
                    for ch in range(2):
                        nc.tensor.matmul(
                            pv[:, ch * 128 : (ch + 1) * 128],
                            lhsT=vt[:, ch, lsub * 128 : (lsub + 1) * 128],
                            rhs=ident16[:],
                            is_transpose=True,
                            start=(ch == 0),
                            stop=(ch == 1),
                        )
                    nc.vector.tensor_copy(
                        Vp16[blk][:, lsub, :, 0:DK],
                        pv[:].rearrange("p (h d) -> p h d", h=GH),
                    )

            def normalize(h, qt, pso):
                hj, hp = h // 2, 64 * (h % 2)
                den_r = cp.tile([1, 512], F32R, tag="den", bufs=2)
                nc.vector.tensor_copy(den_r[:], pso[64:65, :])
                psb = psum.tile([128, 512], F32, tag="ps")
                nc.tensor.matmul(
                    psb[:64], lhsT=ones_r[:, 0:64], rhs=den_r[:], start=True, stop=True
                )
                rb = cp.tile([64, 512], F32, tag="rb", bufs=2)
                nc.vector.reciprocal_approx_fast(rb[:], psb[:64])
                nc.vector.tensor_tensor(
                    OTs[qt][hp : hp + 64, hj, :],
                    pso[:64],
                    rb[:],
                    mybir.AluOpType.mult,
                )

            def outproj(qt512, subs=(0, 1, 2, 3)):
                # project q rows [qt512*512, +512) and DMA them out; woven
                # into the next q-tile's attention so PE stays dense and the
                # output DMA (and its DVE copy burst) is spread out.
                for sub in subs:
                    q0 = qt512 * 512 + sub * 128
                    for e in range(2):
                        psy = psum.tile([128, 512], F32, tag="ps")
                        for cj in range(2):
                            nc.tensor.matmul(
                                psy[:],
                                lhsT=OTs[qt512][:, cj, sub * 128 : (sub + 1) * 128],
                                rhs=WoT[:, cj, e * 512 : (e + 1) * 512],
                                start=(cj == 0),
                                stop=(cj == 1),
                            )
                        y_sb = yp.tile([128, 512], F32, tag="y")
                        nc.vector.tensor_copy(y_sb[:], psy[:])
                        nc.sync.dma_start(
                            out_d[q0 : q0 + 128, e * 512 : (e + 1) * 512], y_sb[:]
                        )

            y3 = {}
            y8 = ctx.enter_context(tc.tile_pool(name="ytail", bufs=8))

            def outproj_half(qt512, cj):
                # last-block outproj split by contraction half: cj=0 runs
                # during the final attention head (PE otherwise waits on
                # exp), cj=1 + accumulate + DMA is all that's left after
                # the last normalize.
                for sub in range(4):
                    for e in range(2):
                        idx = sub * 2 + e
                        psy = psum.tile([128, 512], F32, tag="ps")
                        nc.tensor.matmul(
                            psy[:],
                            lhsT=OTs[qt512][:, cj, sub * 128 : (sub + 1) * 128],
                            rhs=WoT[:, cj, e * 512 : (e + 1) * 512],
                            start=True,
                            stop=True,
                        )
                        if cj == 0:
                            y_sb = y8.tile([128, 512], F32, tag="y8")
                            nc.vector.tensor_copy(y_sb[:], psy[:])
                            y3[idx] = y_sb
                        else:
                            y_sb = y3[idx]
                            nc.vector.tensor_tensor(
                                y_sb[:], y_sb[:], psy[:], mybir.AluOpType.add
                            )
                            q0 = qt512 * 512 + sub * 128
                            nc.sync.dma_start(
                                out_d[q0 : q0 + 128, e * 512 : (e + 1) * 512], y_sb[:]
                            )

            pending = None

            def attn(qt):
                nonlocal pending
                n_kt = 4 * qt + 4
                for h in range(GH):
                    hj = h // 2
                    pso = psum_o.tile([128, 512], F32, tag="pso")
                    for kt2 in range(n_kt // 2):  # k-tile pairs share a
                        pss = psum2.tile([128, 1024], F32, tag="ps2")  # 2-bank psum
                        for i in range(2):
                            kt = 2 * kt2 + i
                            nc.tensor.matmul(
                                pss[:, i * 512 : (i + 1) * 512],
                                lhsT=KTzs[kt // 4][
                                    :, h, (kt % 4) * 128 : (kt % 4 + 1) * 128
                                ],
                                rhs=QTs[qt][:, hj, :],
                                start=True,
                                stop=True,
                            )
                        p_sb = cp.tile([128, 1024], F16, tag="p", bufs=5)
                        # one activation per pair, but start at the first
                        # valid column of the pair's first k-tile: for the
                        # (i=2,3) diagonal pair this trims 256 columns while
                        # keeping the single-exp pipeline rhythm (the
                        # mid-range garbage it still covers is never read)
                        e0 = 128 * max(0, 2 * kt2 - 4 * qt)
                        nc.scalar.activation(
                            p_sb[:, e0:],
                            pss[:, e0:],
                            mybir.ActivationFunctionType.Exp,
                            scale=0.125,
                        )
                        for i in range(2):
                            kt = 2 * kt2 + i
                            if kt >= 4 * qt:  # diagonal tile: causal mask via
                                # a DVE multiply (keeps the Pool engine off
                                # the exp -> AV latency chain)
                                d0 = (kt - 4 * qt) * 128
                                nc.vector.tensor_tensor(
                                    p_sb[:, i * 512 + d0 : i * 512 + d0 + 128],
                                    p_sb[:, i * 512 + d0 : i * 512 + d0 + 128],
                                    tri_mask[:],
                                    mybir.AluOpType.mult,
                                )
                            # AV restricted to valid q columns; cols below
                            # the diagonal tile are never read so the exp
                            # garbage there is harmless
                            d0 = max(0, (kt - 4 * qt) * 128)
                            nc.tensor.matmul(
                                pso[:65, d0:],
                                lhsT=Vp16[kt // 4][:, kt % 4, h, :],
                                rhs=p_sb[:, i * 512 + d0 : (i + 1) * 512],
                                start=(kt == 0),
                                stop=(kt == n_kt - 1),
                                skip_group_check=True,
                            )
                        if kt2 == 0 and pending is not None:
                            normalize(*pending)  # previous tile, PE has work
                            pending = None
                    pending = (h, qt, pso)
                    if h == 0 and qt > 0:
                        outproj(qt - 1, subs=(0, 1))  # prev block normalized
                    if h == 1 and qt > 0:
                        outproj(qt - 1, subs=(2, 3))
                    if h == 2 and qt == QT_TILES - 1:
                        outproj_half(qt, 0)  # heads 0/1 already normalized

            # ==== interleaved: per 512-row block, projections then attention,
            # with the next blocks' x load/cast/transpose woven in
            for blk in range(QT_TILES):
                # next block's x casts + XBAR transposes first: the casts
                # only wait on loads, so DVE drains them before the proj
                # psum reads and the transposes land well before block+1
                if blk < 3:
                    for qt in range(4 + 4 * blk, 8 + 4 * blk):
                        cast_x(qt)
                    for qt in range(4 + 4 * blk, 8 + 4 * blk):
                        xpose_x(qt)
                qk_proj(blk)
                if blk == 0:
                    wt_pe("v", WT["v"], 2)
                if blk == 1:
                    wt_pe("o", WoT, 8)
                v_proj(blk)
                if blk < 2:
                    for qt in range(8 + 4 * blk, 12 + 4 * blk):
                        load_x(qt)
                attn(blk)
            normalize(*pending)
            outproj_half(QT_TILES - 1, 1)

    nc.compile()
    return nc


_NC_CACHE = None


def _get_program():
    global _NC_CACHE
    if _NC_CACHE is None:
        _NC_CACHE = _build_program()
    return _NC_CACHE


def _run(in_maps, trace=False, **kw):
    nc = _get_program()
    return run_bass_kernel_spmd(nc, in_maps, list(range(NCORES)), trace=trace, **kw)


def _make_in_maps(x, Wq, bq, Wk, bk, Wv, bv, Wo, bo):
    a = lambda v: np.ascontiguousarray(np.asarray(v, dtype=np.float32))
    in_maps = []
    for core in range(NCORES):
        b, g = divmod(core, 4)
        s = slice(g * C, (g + 1) * C)
        in_maps.append(
            {
                "x": a(x[b]),
                "wq": a(Wq[s, :]),
                "wk": a(Wk[s, :]),
                "wv": a(Wv[s, :]),
                "wo": a(Wo[:, s]),
                "bq": a(bq[s]),
                "bk": a(bk[s]),
                "bv": a(bv[s]),
            }
        )
    return in_maps


def kernel(x, Wq, bq, Wk, bk, Wv, bv, Wo, bo, _trace=False, _trace_out=None, _tmpdir=None):
    in_maps = _make_in_maps(x, Wq, bq, Wk, bk, Wv, bv, Wo, bo)
    res = _run(in_maps, trace=_trace, tmpdir=_tmpdir)
    if _trace_out is not None:
        _trace_out.append(res)
    bo = np.asarray(bo, dtype=np.float32)
    out = np.empty((B, L, D), dtype=np.float32)
    for b in range(B):
        acc = res.results[4 * b]["out"].astype(np.float32)
        for g in range(1, 4):
            acc = acc + res.results[4 * b + g]["out"]
        out[b] = acc + bo[None, :]
    return out


# revision 73
# speedup vs baseline: 1.0308x; 1.0051x over previous
"""Multi-head causal self-attention (B=2, L=2048, D=1024, H=16) on 8 TRN2
NeuronCores.  ~218 us HW exec (baseline 273-315 us).

Sharding: core c handles batch b = c // 4 and head group g = c % 4 (4 heads,
i.e. a 256-wide slice of the QKV output dim and the matching 256 rows of
Wo^T).  Each core computes a full (L, D) partial of the output projection;
the host sums the 4 partials per batch and adds bo.

All matmul operands are bf16/f16 (accumulation f32 in PSUM): vs the f32r
original this halves LDWEIGHTS (which was rate-limiting at 229 ns vs the
213 ns per-512-column matmul stream) and makes every matmul stream at the
full 2.4 GHz / 1 column/cycle.

On-core layout:
  XT  [128, 8, 2048]   x^T (d-chunk on partitions).  x tiles 0-3 transpose
                       on the PE during the startup DMA window (also ramps
                       the HAM clock); tiles 4-15 go f32 load -> Pool cast
                       to bf16 -> XBAR DMA transpose (14 ns/16x128 tile on
                       the otherwise-idle DMA engines), software-pipelined
                       one 512-row block ahead of consumption.
  WqT/WkT/WvT [128, 8, 256], WoT [128, 2, 1024]  W^T via PE transposes in
                       the same startup window.
  QT  [128, 2, 2048]   q^T (dq on partitions, chunk = head pair)
  KTz [128, 4, 2048]   k^T zero-padded per head to K=128 rows: the PE HAM
                       clock gate only un-throttles (1.2 -> 2.4 GHz) when
                       matmuls stream all 128 partitions.
  Vp  [128, 4, 4, 65]  v natural + ones column (softmax denominator trick),
                       built from 512-col v^T matmuls (weight loads stay
                       hidden) + f16 PE transposes back to natural layout.
  OT  [128, 2, 2048]   attention out^T, normalized in place

DMA choreography matters as much as compute: one HWDGE ring's transfers
are serial in issue order, so the sync ring is hand-ordered (x0-3 + w
loads, then per-block xbar transposes ahead of the next block's loads,
y stores last) and the tiny bias loads ride the scalar ring.

Projections and attention are interleaved per 512-row q block.  Attention
per (qt, head): s^T[k, q] = KTz_h . QT_pair; exp on ACT from a 2-bank PSUM
pair (ACT is the attention-phase pacer at ~89 us total); causal mask via a
DVE multiply with a precomputed 128x128 triangle (keeps the Pool engine
off the exp -> AV latency chain), with AV column-trimmed to the valid q
range on diagonal tiles; o^T + denominator accumulated in PSUM with V';
normalize = PE ones-broadcast of the denominator + DVE
reciprocal_approx_fast (5x faster than reciprocal(); the denominator is a
sum of exps, far from the undefined edge cases) + one multiply, emitted
one tile late so the PE stream never waits.  The output projection is
woven in per 512-row q block, and the last block's is split by
contraction half so only half of it (plus adds and stores) trails the
final normalize.

Things measured NOT to work here: f32r anywhere (LDWEIGHTS-bound),
software-DGE cast DMAs (~7x slower than HWDGE), loads on the scalar ring
(starved), fp8 DoubleRow AV (mixing DoubleRow and normal matmuls in one
PSUM accumulation group corrupts it -> NaN; pure-DR works but early
causal rows then fail the precision gate), scalar-engine psum->sbuf
copies during flight (ACT in-order execution stalls the exp stream), and
causal triangle trimming of scores/exp (short matmul streams expose
LDWEIGHTS and break the pair-pipeline rhythm).
"""

import sys

for _p in ("/opt/trn_rl_repo", "/root/.axon_site/_ro/trn_rl_repo"):
    if _p not in sys.path:
        sys.path.append(_p)

from contextlib import ExitStack

import numpy as np

import concourse.bass as bass
import concourse.tile as tile
from concourse import bacc, mybir
from concourse.bass_utils import run_bass_kernel_spmd
from concourse.masks import make_identity

F32 = mybir.dt.float32
F32R = mybir.dt.float32r
F16 = mybir.dt.float16
BF16 = mybir.dt.bfloat16
F8E4 = mybir.dt.float8e4

B, L, D, H = 2, 2048, 1024, 16
DK = D // H  # 64
NCORES = 8
GH = 4  # heads per core
C = GH * DK  # 256: per-core slice of the qkv/head dim
QT_TILES = L // 512  # 4
KT_TILES = L // 128  # 16
DCH = D // 128  # 8


def _build_program():
    nc = bacc.Bacc("TRN2", target_bir_lowering=False, debug=False, num_devices=NCORES)

    x_d = nc.dram_tensor("x", [L, D], F32, kind="ExternalInput").ap()
    wq_d = nc.dram_tensor("wq", [C, D], F32, kind="ExternalInput").ap()
    wk_d = nc.dram_tensor("wk", [C, D], F32, kind="ExternalInput").ap()
    wv_d = nc.dram_tensor("wv", [C, D], F32, kind="ExternalInput").ap()
    wo_d = nc.dram_tensor("wo", [D, C], F32, kind="ExternalInput").ap()
    bq_d = nc.dram_tensor("bq", [C], F32, kind="ExternalInput").ap()
    bk_d = nc.dram_tensor("bk", [C], F32, kind="ExternalInput").ap()
    bv_d = nc.dram_tensor("bv", [C], F32, kind="ExternalInput").ap()
    out_d = nc.dram_tensor("out", [L, D], F32, kind="ExternalOutput").ap()

    with tile.TileContext(nc) as tc, ExitStack() as ctx:
        pool = ctx.enter_context(tc.tile_pool(name="persist", bufs=1))
        psum = ctx.enter_context(tc.tile_pool(name="psum", bufs=2, space="PSUM"))
        psum_o = ctx.enter_context(tc.tile_pool(name="psum_o", bufs=2, space="PSUM"))
        psum2 = ctx.enter_context(tc.tile_pool(name="psum2", bufs=2, space="PSUM"))
        cp = ctx.enter_context(tc.tile_pool(name="copies", bufs=3))
        yp = ctx.enter_context(tc.tile_pool(name="youts", bufs=2))

        ones_f32 = pool.tile([1, 128], F32)
        nc.gpsimd.memset(ones_f32[:], 1.0)
        ones_r = pool.tile([1, 128], F32R)
        nc.vector.tensor_copy(ones_r[:], ones_f32[:])
        ident = pool.tile([128, 128], F32)
        make_identity(nc, ident)
        ident16 = pool.tile([128, 128], F16)
        nc.vector.tensor_copy(ident16[:], ident[:])
        tri_mask = pool.tile([128, 128], F16)
        nc.gpsimd.memset(tri_mask[:], 1.0)
        nc.gpsimd.affine_select(
            out=tri_mask[:],
            in_=tri_mask[:],
            pattern=[[1, 128]],
            compare_op=mybir.AluOpType.is_ge,
            fill=0.0,
            base=0,
            channel_multiplier=-1,
        )

        bq_sb = pool.tile([128, 2], F32)
        nc.scalar.dma_start(bq_sb[:], bq_d.rearrange("(c p) -> p c", p=128))
        bk_sb = pool.tile([128, 2], F32)
        nc.scalar.dma_start(bk_sb[:], bk_d.rearrange("(c p) -> p c", p=128))
        bv_sb = pool.tile([128, 2], F32)
        nc.scalar.dma_start(bv_sb[:], bv_d.rearrange("(c p) -> p c", p=128))

        QTs = [pool.tile([128, 2, 512], F16, name=f"QT{g}") for g in range(4)]
        KTzs = [pool.tile([128, GH, 512], F16, name=f"KTz{g}") for g in range(4)]
        Vp16 = [pool.tile([128, 4, GH, DK + 1], F16, name=f"Vp16_{g}") for g in range(4)]
        OTs = [pool.tile([128, 2, 512], BF16, name=f"OT{g}") for g in range(4)]

        # ---- weight + first-block x transposes on the PE (f32 in, psum,
        # DVE copy casts to bf16) during the startup DMA window; x4-15 go
        # HWDGE f32 load -> DVE cast -> XBAR DMA transpose on the sync
        # ring so the PE never pays for them. ----
        XT = pool.tile([128, DCH, L], BF16)
        WT = {}
        lp = ctx.enter_context(tc.tile_pool(name="loads", bufs=6))
        cb = ctx.enter_context(tc.tile_pool(name="casts", bufs=3))
        xsbs, xbs = {}, {}

        def load_x(qt, split=False):
            # split=True loads in two halves so the PE transpose of the
            # first 4 d-chunks can start as soon as half the tile lands
            xsb = lp.tile([128, D], F32, tag="xsb")
            if split:
                for hh in range(2):
                    nc.sync.dma_start(
                        xsb[:, hh * 512 : (hh + 1) * 512],
                        x_d[qt * 128 : (qt + 1) * 128, hh * 512 : (hh + 1) * 512],
                    )
            else:
                nc.sync.dma_start(xsb[:], x_d[qt * 128 : (qt + 1) * 128, :])
            xsbs[qt] = xsb

        def cast_x(qt):
            # Pool engine: ~2.5x slower per copy than DVE but completely off
            # the DVE relay chain (proj psum -> QT/KTz -> scores) that paces
            # each block boundary; transposes still land with slack to spare
            xb = cb.tile([128, D], BF16, tag="xb")
            nc.gpsimd.tensor_copy(xb[:], xsbs.pop(qt)[:])
            xbs[qt] = xb

        def xpose_x(qt):
            nc.sync.dma_start(
                XT[:, :, qt * 128 : (qt + 1) * 128], xbs.pop(qt)[:], transpose=True
            )

        wsbs = {}

        def load_w(name, w_d):
            wsb = lp.tile([128, 2048], F32, tag="wsb", bufs=4)
            nc.sync.dma_start(
                wsb[:].rearrange("p (c d) -> p c d", c=w_d.shape[0] // 128),
                w_d.rearrange("(c p) d -> p c d", p=128),
            )
            wsbs[name] = wsb

        def _transpose_block(dst, src, n_chunks, col0):
            """PE-transpose n_chunks 128x128 f32 blocks of src into
            dst[:, chunk, col0:col0+128] (DVE copy casts to dst dtype)."""
            for half in range(0, n_chunks, 4):
                n = min(4, n_chunks - half)
                pt = psum.tile([128, 512], F32, tag="ps", name="pt")
                for i in range(n):
                    dci = half + i
                    nc.tensor.matmul(
                        pt[:, i * 128 : (i + 1) * 128],
                        lhsT=src[:, dci * 128 : (dci + 1) * 128],
                        rhs=ident[:],
                        is_transpose=True,
                        start=(i == 0),
                        stop=(i == n - 1),
                    )
                nc.vector.tensor_copy(
                    dst[:, half : half + n, col0 : col0 + 128],
                    pt[:, : n * 128].rearrange("p (c q) -> p c q", c=n),
                )

        def xpose_pe_x(qt):
            _transpose_block(XT, xsbs.pop(qt)[:], DCH, qt * 128)

        def wt_pe(name, dst, outer):
            # wsb [128, outer, inner*128]; chunk r transposes into
            # dst[:, :, r*128:(r+1)*128]
            wsb = wsbs.pop(name)[:].rearrange("p (c d) -> p c d", c=outer)
            for r in range(outer):
                _transpose_block(dst, wsb[:, r, :], 2048 // (128 * outer), r * 128)

        # prologue: x0-3 + wq/wk loads, PE transposes fill the startup
        for qt in range(4):
            load_x(qt, split=True)
        load_w("q", wq_d)
        load_w("k", wk_d)
        for name in ("q", "k", "v"):
            WT[name] = pool.tile([128, DCH, C], BF16, name=f"W{name}T")
        WoT = pool.tile([128, 2, D], BF16)
        for qt in range(4):
            xpose_pe_x(qt)
        wt_pe("q", WT["q"], 2)
        wt_pe("k", WT["k"], 2)
        load_w("v", wv_d)
        for qt in range(4, 6):
            load_x(qt)
        load_w("o", wo_d)
        for qt in range(6, 8):
            load_x(qt)
        # zero-fills: Pool memsets + small DVE copies, off the cast path
        for g in range(4):
            for h in range(GH):
                zp = 64 - 64 * (h % 2)
                nc.gpsimd.memset(KTzs[g][zp : zp + 64, h, :], 0.0)
        for g in range(4):
            nc.gpsimd.memset(Vp16[g][:, :, :, DK], 1.0)

        with nc.allow_low_precision(reason="bf16/f16 matmul inputs"):

            def qk_proj(blk):
                # j-major, k first: attention h0 needs only the k-j0 and
                # q-j0 outputs, so it can start after two psums, not three
                for j in range(2):
                    for name in ("k", "q"):
                        wt = WT[name]
                        b_sb = bq_sb if name == "q" else bk_sb
                        ps = psum.tile([128, 512], F32, tag="ps")
                        for dci in range(DCH):
                            nc.tensor.matmul(
                                ps[:],
                                lhsT=wt[:, dci, j * 128 : (j + 1) * 128],
                                rhs=XT[:, dci, blk * 512 : (blk + 1) * 512],
                                start=(dci == 0),
                                stop=(dci == DCH - 1),
                            )
                        if name == "q":
                            nc.vector.tensor_tensor(
                                QTs[blk][:, j, :],
                                ps[:],
                                b_sb[:, j, None].to_broadcast((128, 512)),
                                mybir.AluOpType.add,
                            )
                        else:
                            # write k^T split into the zero-padded KTz
                            for half in range(2):
                                hp = 64 * half
                                nc.vector.tensor_tensor(
                                    KTzs[blk][hp : hp + 64, 2 * j + half, :],
                                    ps[hp : hp + 64, :],
                                    b_sb[hp : hp + 64, j, None].to_broadcast(
                                        (64, 512)
                                    ),
                                    mybir.AluOpType.add,
                                )

            def v_proj(blk):
                # v^T via 512-col matmuls (weight loads stay hidden), bias
                # added per-partition on the psum read, then f16 PE
                # transposes back into the natural-layout Vp
                vt = cb.tile([128, 2, 512], F16, tag="vt", bufs=2)
                for ch in range(2):
                    ps = psum.tile([128, 512], F32, tag="ps")
                    for dci in range(DCH):
                        nc.tensor.matmul(
                            ps[:],
                            lhsT=WT["v"][:, dci, ch * 128 : (ch + 1) * 128],
                            rhs=XT[:, dci, blk * 512 : (blk + 1) * 512],
                            start=(dci == 0),
                            stop=(dci == DCH - 1),
                        )
                    nc.vector.tensor_tensor(
                        vt[:, ch, :],
                        ps[:],
                        bv_sb[:, ch, None].to_broadcast((128, 512)),
                        mybir.AluOpType.add,
                    )
                for lsub in range(4):
                    pv = psum.tile([128, 256], F16, tag="ps")
                    for ch in range(2):
                        nc.tensor.matmul(
                            pv[:, ch * 128 : (ch + 1) * 128],
                            lhsT=vt[:, ch, lsub * 128 : (lsub + 1) * 128],
                            rhs=ident16[:],
                            is_transpose=True,
                            start=(ch == 0),
                            stop=(ch == 1),
                        )
                    nc.vector.tensor_copy(
                        Vp16[blk][:, lsub, :, 0:DK],
                        pv[:].rearrange("p (h d) -> p h d", h=GH),
                    )

            def normalize(h, qt, pso):
                hj, hp = h // 2, 64 * (h % 2)
                den_r = cp.tile([1, 512], F32R, tag="den", bufs=2)
                nc.vector.tensor_copy(den_r[:], pso[64:65, :])
                psb = psum.tile([128, 512], F32, tag="ps")
                nc.tensor.matmul(
                    psb[:64], lhsT=ones_r[:, 0:64], rhs=den_r[:], start=True, stop=True
                )
                rb = cp.tile([64, 512], F32, tag="rb", bufs=2)
                nc.vector.reciprocal_approx_fast(rb[:], psb[:64])
                nc.vector.tensor_tensor(
                    OTs[qt][hp : hp + 64, hj, :],
                    pso[:64],
                    rb[:],
                    mybir.AluOpType.mult,
                )

            def outproj(qt512, subs=(0, 1, 2, 3)):
                # project q rows [qt512*512, +512) and DMA them out; woven
                # into the next q-tile's attention so PE stays dense and the
                # output DMA (and its DVE copy burst) is spread out.
                for sub in subs:
                    q0 = qt512 * 512 + sub * 128
                    for e in range(2):
                        psy = psum.tile([128, 512], F32, tag="ps")
                        for cj in range(2):
                            nc.tensor.matmul(
                                psy[:],
                                lhsT=OTs[qt512][:, cj, sub * 128 : (sub + 1) * 128],
                                rhs=WoT[:, cj, e * 512 : (e + 1) * 512],
                                start=(cj == 0),
                                stop=(cj == 1),
                            )
                        y_sb = yp.tile([128, 512], F32, tag="y")
                        nc.vector.tensor_copy(y_sb[:], psy[:])
                        nc.sync.dma_start(
                            out_d[q0 : q0 + 128, e * 512 : (e + 1) * 512], y_sb[:]
                        )

            y3 = {}
            y8 = ctx.enter_context(tc.tile_pool(name="ytail", bufs=8))

            def outproj_half(qt512, cj, subs=(0, 1, 2, 3)):
                # last-block outproj split by contraction half: cj=0 runs
                # during the final attention heads (PE otherwise waits on
                # exp), cj=1 + accumulate + DMA is all that's left after
                # the last normalize.
                for sub in subs:
                    for e in range(2):
                        idx = sub * 2 + e
                        psy = psum.tile([128, 512], F32, tag="ps")
                        nc.tensor.matmul(
                            psy[:],
                            lhsT=OTs[qt512][:, cj, sub * 128 : (sub + 1) * 128],
                            rhs=WoT[:, cj, e * 512 : (e + 1) * 512],
                            start=True,
                            stop=True,
                        )
                        if cj == 0:
                            y_sb = y8.tile([128, 512], F32, tag="y8")
                            nc.vector.tensor_copy(y_sb[:], psy[:])
                            y3[idx] = y_sb
                        else:
                            y_sb = y3[idx]
                            nc.vector.tensor_tensor(
                                y_sb[:], y_sb[:], psy[:], mybir.AluOpType.add
                            )
                            q0 = qt512 * 512 + sub * 128
                            nc.sync.dma_start(
                                out_d[q0 : q0 + 128, e * 512 : (e + 1) * 512], y_sb[:]
                            )

            pending = None

            def attn(qt):
                nonlocal pending
                n_kt = 4 * qt + 4
                for h in range(GH):
                    hj = h // 2
                    pso = psum_o.tile([128, 512], F32, tag="pso")
                    for kt2 in range(n_kt // 2):  # k-tile pairs share a
                        pss = psum2.tile([128, 1024], F32, tag="ps2")  # 2-bank psum
                        for i in range(2):
                            kt = 2 * kt2 + i
                            nc.tensor.matmul(
                                pss[:, i * 512 : (i + 1) * 512],
                                lhsT=KTzs[kt // 4][
                                    :, h, (kt % 4) * 128 : (kt % 4 + 1) * 128
                                ],
                                rhs=QTs[qt][:, hj, :],
                                start=True,
                                stop=True,
                            )
                        p_sb = cp.tile([128, 1024], F16, tag="p", bufs=5)
                        # one activation per pair, but start at the first
                        # valid column of the pair's first k-tile: for the
                        # (i=2,3) diagonal pair this trims 256 columns while
                        # keeping the single-exp pipeline rhythm (the
                        # mid-range garbage it still covers is never read)
                        e0 = 128 * max(0, 2 * kt2 - 4 * qt)
                        nc.scalar.activation(
                            p_sb[:, e0:],
                            pss[:, e0:],
                            mybir.ActivationFunctionType.Exp,
                            scale=0.125,
                        )
                        for i in range(2):
                            kt = 2 * kt2 + i
                            if kt >= 4 * qt:  # diagonal tile: causal mask via
                                # a DVE multiply (keeps the Pool engine off
                                # the exp -> AV latency chain)
                                d0 = (kt - 4 * qt) * 128
                                nc.vector.tensor_tensor(
                                    p_sb[:, i * 512 + d0 : i * 512 + d0 + 128],
                                    p_sb[:, i * 512 + d0 : i * 512 + d0 + 128],
                                    tri_mask[:],
                                    mybir.AluOpType.mult,
                                )
                            # AV restricted to valid q columns; cols below
                            # the diagonal tile are never read so the exp
                            # garbage there is harmless
                            d0 = max(0, (kt - 4 * qt) * 128)
                            nc.tensor.matmul(
                                pso[:65, d0:],
                                lhsT=Vp16[kt // 4][:, kt % 4, h, :],
                                rhs=p_sb[:, i * 512 + d0 : (i + 1) * 512],
                                start=(kt == 0),
                                stop=(kt == n_kt - 1),
                                skip_group_check=True,
                            )
                        if kt2 == 0 and pending is not None:
                            normalize(*pending)  # previous tile, PE has work
                            pending = None
                    pending = (h, qt, pso)
                    if qt > 0:
                        outproj(qt - 1, subs=(h,))  # prev block normalized
                    if qt == QT_TILES - 1:
                        if h == 2:
                            outproj_half(qt, 0, subs=(0, 1))  # heads 0/1 done
                        elif h == 3:
                            outproj_half(qt, 0, subs=(2, 3))

            # ==== interleaved: per 512-row block, projections then attention,
            # with the next blocks' x load/cast/transpose woven in
            for blk in range(QT_TILES):
                # next block's x casts + XBAR transposes first: the casts
                # only wait on loads, so DVE drains them before the proj
                # psum reads and the transposes land well before block+1
                if blk < 3:
                    for qt in range(4 + 4 * blk, 8 + 4 * blk):
                        cast_x(qt)
                    for qt in range(4 + 4 * blk, 8 + 4 * blk):
                        xpose_x(qt)
                qk_proj(blk)
                if blk == 0:
                    wt_pe("v", WT["v"], 2)
                if blk == 1:
                    wt_pe("o", WoT, 8)
                v_proj(blk)
                if blk < 2:
                    for qt in range(8 + 4 * blk, 12 + 4 * blk):
                        load_x(qt)
                attn(blk)
            normalize(*pending)
            outproj_half(QT_TILES - 1, 1)

    nc.compile()
    return nc


_NC_CACHE = None


def _get_program():
    global _NC_CACHE
    if _NC_CACHE is None:
        _NC_CACHE = _build_program()
    return _NC_CACHE


def _run(in_maps, trace=False, **kw):
    nc = _get_program()
    return run_bass_kernel_spmd(nc, in_maps, list(range(NCORES)), trace=trace, **kw)


def _make_in_maps(x, Wq, bq, Wk, bk, Wv, bv, Wo, bo):
    a = lambda v: np.ascontiguousarray(np.asarray(v, dtype=np.float32))
    in_maps = []
    for core in range(NCORES):
        b, g = divmod(core, 4)
        s = slice(g * C, (g + 1) * C)
        in_maps.append(
            {
                "x": a(x[b]),
                "wq": a(Wq[s, :]),
                "wk": a(Wk[s, :]),
                "wv": a(Wv[s, :]),
                "wo": a(Wo[:, s]),
                "bq": a(bq[s]),
                "bk": a(bk[s]),
                "bv": a(bv[s]),
            }
        )
    return in_maps


def kernel(x, Wq, bq, Wk, bk, Wv, bv, Wo, bo, _trace=False, _trace_out=None, _tmpdir=None):
    in_maps = _make_in_maps(x, Wq, bq, Wk, bk, Wv, bv, Wo, bo)
    res = _run(in_maps, trace=_trace, tmpdir=_tmpdir)
    if _trace_out is not None:
        _trace_out.append(res)
    bo = np.asarray(bo, dtype=np.float32)
    out = np.empty((B, L, D), dtype=np.float32)
    for b in range(B):
        acc = res.results[4 * b]["out"].astype(np.float32)
        for g in range(1, 4):
            acc = acc + res.results[4 * b + g]["out"]
        out[b] = acc + bo[None, :]
    return out
